# revision 17
# baseline (speedup 1.0000x reference)
"""Trainium2 Bass kernel for EncodecEuclideanCodebook (VQ encode+decode).

Problem: hidden_states [8, 32768, 128] f32, embed [1024, 128] f32.
  scores = x @ E^T - ||e||^2/2   (argmax-equivalent to reference dist)
  embed_ind = argmax(scores, axis=-1)           -> [8, 32768] int32
  quantize  = embed[embed_ind]                  -> [8, 32768, 128] f32

Sharding: data-parallel over tokens; core i handles hidden_states[i]
(32768 tokens). embed replicated. No collectives.

Per-core pipeline, 256 tiles of 128 tokens:
  DMA x tile -> PE transpose (x^T) -> fp32 matmuls vs resident E^T
  (+ bf16x3-decomposed -e^2/2 bias row matmul) -> scores in PSUM
  -> ACT copy PSUM->SBUF -> GPSIMD max-reduce (or DVE InstMax for a
  fraction of tiles, to balance engines) -> DVE max_index -> idx
  -> GPSIMD indirect-DMA gather embed[idx] -> DMA out.

Exactness notes (measured on the seed-0 data offline):
  min top-2 score gap = 2.1e-5, so bf16/tf32 GEMM or any quantized
  argmax packing flips indices; full-fp32 GEMM and exact fp32
  compares are required. jnp.argmax picks the first max; InstMaxIndex
  also returns the first match.
"""

import os
import sys

sys.path.insert(0, "/opt/trn_rl_repo")

import numpy as np

import concourse.bass as bass
import concourse.mybir as mybir
import concourse.tile as tile
from concourse import bacc
from concourse.bass import IndirectOffsetOnAxis
from concourse.masks import make_identity

# ---------------------------------------------------------------------------
# Custom DVE ops (registered into concourse's OPS table at import time).
#
# VQ_PAIRMAX_RED: out[k] = max(in0[k], in1[k]); accum_out = row max.
#   One 512-elem pass examines all 1024 scores (2 streams) and yields the
#   exact fp32 row max.
# VQ_ARGMIN_HIT: cand[k] = (k+1) + (in0[k] < m) * BIG; accum_out = min cand
#   = 1 + first index where in0 == m (first occurrence, matching jnp.argmax).
#   BIG = 1e9 via s1 (also the accum seed).
# ---------------------------------------------------------------------------
from concourse import dve_ops as _dve_ops
from concourse.dve_spec import (
    Spec as _Spec, Src0 as _Src0, Src1 as _Src1, C0 as _C0, C1 as _C1,
    maxx as _maxx, minn as _minn, scan as _scan, AluOp as _AluOp, One as _One,
    lower as _lower,
)
from concourse.dve_uop import DveOpSpec as _DveOpSpec
import numpy as _np


def _ref_pairmax(in0, in1, c0, c1, c2):
    b = _np.maximum(in0, in1).astype(_np.float32)
    return b, b.reshape(b.shape[0], -1).max(axis=-1, keepdims=True)


def _ref_argmin_hit(in0, in1, c0, c1, c2):
    n = in0.shape[-1]
    pos = _np.arange(1, n + 1, dtype=_np.float32).reshape(
        (1,) * (in0.ndim - 1) + (n,))
    cand = (pos + (in0 < c0) * c1).astype(_np.float32)
    flat = cand.reshape(cand.shape[0], -1)
    acc = _np.minimum(flat.min(axis=-1, keepdims=True),
                      _np.asarray(c1, _np.float32).reshape(-1, 1)
                      if isinstance(c1, _np.ndarray) else c1)
    return cand, acc


def _register_op(name, spec, rd1_en):
    existing = {o.name: o for o in _dve_ops.OPS}
    if name in existing:
        return existing[name]
    op = _dve_ops.DveOp(name, spec, subdim=False, uops_sha={})
    _dve_ops.OPS.append(op)
    _dve_ops.CUSTOM_DVE_SPECS[name] = spec
    _dve_ops._SUB_OPCODE_FOR_NAME[name] = (
        _dve_ops._CUSTOM_DVE_ROW_BASE + len(_dve_ops.OPS) - 1)
    assert _dve_ops._SUB_OPCODE_FOR_NAME[name] < 0x20
    shas = {}
    for ver in ("v3", "v4"):
        u = _lower(spec, ver=ver)
        shas[ver] = _DveOpSpec(name=name, opcode=_dve_ops.get_dve_sub_opcode(name),
                               uops=u, rd1_en=rd1_en).sha(ver)
    object.__setattr__(op, "uops_sha", shas)
    return op


VQ_PAIRMAX_RED = _register_op(
    "VQ_PAIRMAX_RED",
    _Spec(body=_maxx(_Src0, _Src1), accum=_maxx, reference=_ref_pairmax),
    rd1_en=True)

VQ_ARGMIN_HIT = _register_op(
    "VQ_ARGMIN_HIT",
    _Spec(body=_scan(_AluOp.ADD, _One) + (_Src0 < _C0) * _C1,
          accum=_minn, accum_init=_C1, reference=_ref_argmin_hit),
    rd1_en=False)

P = 128
BINS = 1024
DIM = 128
N_CORES = 8

# Tiles with (t % DVE_MAX_EVERY == DVE_MAX_EVERY-1) run the row-max fully on
# DVE (InstMax over 1024); the rest use GPSIMD pairwise-max + DVE InstMax over
# 512. Balances scan work between DVE and GPSIMD. 0 -> all GPSIMD-assisted.
DVE_MAX_EVERY = 0


def build_vq_body(tc, x_ap, embed_ap, quant_ap, idx_ap, n_tokens, repeat=1):
    """Emit the per-core kernel body. x [n_tokens,128] f32, embed [1024,128]
    f32, quant [n_tokens,128] f32, idx [n_tokens] int32 (DRAM APs).
    repeat>1 wraps the main loop in a hardware For_i that redoes the whole
    token sweep (idempotent) -- used only for wall-clock timing deltas."""
    nc = tc.nc
    nt = n_tokens // P  # token tiles
    f32 = mybir.dt.float32
    bf16 = mybir.dt.bfloat16

    from contextlib import ExitStack
    ctx = ExitStack()
    const = ctx.enter_context(tc.tile_pool(name="const", bufs=1))

    # ---------- one-time setup (own PSUM stack, freed before main loop) ----
    setup_ctx = ExitStack()
    psum_su = setup_ctx.enter_context(
        tc.tile_pool(name="psum_su", bufs=1, space="PSUM"))

    identity = const.tile([P, P], f32)
    make_identity(nc, identity[:])

    # E^T resident in SBUF: [dim=128 partitions, 1024 bins]
    et = const.tile([P, BINS], f32)
    for k in range(BINS // P):
        e_blk = const.tile([P, DIM], f32, tag="e_blk")
        nc.sync.dma_start(e_blk[:], embed_ap[k * P:(k + 1) * P, :])
        et_ps = psum_su.tile([P, P], f32, tag="setup_ps")
        nc.tensor.transpose(et_ps[:], e_blk[:], identity[:])
        nc.scalar.copy(et[:, k * P:(k + 1) * P], et_ps[:])

    # e_sq column sums via ones^T @ (E^T * E^T)
    sq = const.tile([P, BINS], f32)
    nc.vector.tensor_tensor(out=sq[:], in0=et[:], in1=et[:],
                            op=mybir.AluOpType.mult)
    ones_col = const.tile([P, 1], f32)
    nc.vector.memset(ones_col[:], 1.0)
    es_sb = const.tile([1, BINS], f32)
    for h in range(2):
        sl = slice(h * 512, (h + 1) * 512)
        es_ps = psum_su.tile([1, 512], f32, tag="es_ps")
        nc.tensor.matmul(es_ps[:], lhsT=ones_col[:], rhs=sq[:, sl],
                         start=True, stop=True)
        nc.scalar.copy(es_sb[0:1, sl], es_ps[:])
    # bias = -e_sq/2, decomposed into 3 bf16 rows (hi+mid+lo ~ exact fp32)
    b_f32 = const.tile([1, BINS], f32)
    nc.vector.tensor_scalar(out=b_f32[:], in0=es_sb[:], scalar1=-0.5,
                            scalar2=None, op0=mybir.AluOpType.mult)
    hi = const.tile([1, BINS], bf16)
    nc.vector.tensor_copy(hi[:], b_f32[:])
    r1 = const.tile([1, BINS], f32)
    nc.vector.tensor_tensor(out=r1[:], in0=b_f32[:], in1=hi[:],
                            op=mybir.AluOpType.subtract)
    mid = const.tile([1, BINS], bf16)
    nc.vector.tensor_copy(mid[:], r1[:])
    r2 = const.tile([1, BINS], f32)
    nc.vector.tensor_tensor(out=r2[:], in0=r1[:], in1=mid[:],
                            op=mybir.AluOpType.subtract)
    lo = const.tile([1, BINS], bf16)
    nc.vector.tensor_copy(lo[:], r2[:])
    bias3 = const.tile([3, BINS], bf16)
    nc.sync.dma_start(bias3[0:1, :], hi[:])
    nc.sync.dma_start(bias3[1:2, :], mid[:])
    nc.sync.dma_start(bias3[2:3, :], lo[:])
    ones3 = const.tile([3, P], bf16)
    nc.vector.memset(ones3[:], 1.0)

    setup_ctx.close()  # release setup PSUM banks

    psum_x = ctx.enter_context(tc.tile_pool(name="psum_x", bufs=2, space="PSUM"))
    psum_s = ctx.enter_context(tc.tile_pool(name="psum_s", bufs=2, space="PSUM"))
    p_xa = ctx.enter_context(tc.tile_pool(name="xa", bufs=3))
    p_xt = ctx.enter_context(tc.tile_pool(name="xt", bufs=3))
    p_s = ctx.enter_context(tc.tile_pool(name="s", bufs=3))
    p_m = ctx.enter_context(tc.tile_pool(name="m", bufs=4))
    p_i = ctx.enter_context(tc.tile_pool(name="i", bufs=4))
    p_q = ctx.enter_context(tc.tile_pool(name="q", bufs=3))

    # idx accumulator: column t holds tile t's indices (as f32, exact ints)
    idx_acc = const.tile([P, nt], f32)

    idx_view = idx_ap.rearrange("(t p) -> t p", p=P)  # [nt, 128]

    rep_cm = tc.For_i(0, repeat, 1) if repeat > 1 else None
    if rep_cm is not None:
        rep_cm.__enter__()

    # ---------- main loop ----------
    for t in range(nt):
        tok = slice(t * P, (t + 1) * P)
        xa = p_xa.tile([P, DIM], f32)
        nc.sync.dma_start(xa[:], x_ap[tok, :])

        xt_ps = psum_x.tile([P, P], f32, tag="xt_ps")
        nc.tensor.transpose(xt_ps[:], xa[:], identity[:])
        xt = p_xt.tile([P, P], f32)
        nc.scalar.copy(xt[:], xt_ps[:])

        s_ps = psum_s.tile([P, BINS], f32)
        for h in range(2):
            sl = slice(h * 512, (h + 1) * 512)
            nc.tensor.matmul(s_ps[:, sl], lhsT=xt[:], rhs=et[:, sl],
                             start=True, stop=False)
            nc.tensor.matmul(s_ps[:, sl], lhsT=ones3[:], rhs=bias3[:, sl],
                             start=False, stop=True)

        # Row max: ACT copies bank1 to SBUF so the custom pairmax op can
        # stream PSUM bank0 + SBUF bank1 (one pass over 512 pairs).
        b1_sb = p_s.tile([P, 512], f32, tag="b1")
        nc.scalar.copy(b1_sb[:], s_ps[:, 512:1024])
        pm = p_s.tile([P, 512], f32, tag="pm")
        m1 = p_m.tile([P, 1], f32, tag="m1")
        nc.vector._custom_dve(VQ_PAIRMAX_RED, out=pm[:], accum_out=m1[:],
                              in0=s_ps[:, 0:512], in1=b1_sb[:])
        # First index where score == max, via min over (pos+1 | miss*1e9).
        am = p_s.tile([P, BINS], f32, tag="am")
        idxp1 = p_m.tile([P, 1], f32, tag="idxp1")
        nc.vector._custom_dve(VQ_ARGMIN_HIT, out=am[:], accum_out=idxp1[:],
                              in0=s_ps[:, 0:BINS], s0=m1[:, 0:1], s1=1e9)
        nc.vector.tensor_copy(idx_acc[:, t:t + 1], idxp1[:, 0:1])  # idx+1

        idx_i = p_i.tile([P, 1], mybir.dt.int32)
        nc.vector.tensor_scalar(out=idx_i[:], in0=idxp1[:], scalar1=-1.0,
                                scalar2=None, op0=mybir.AluOpType.add)

        q_sb = p_q.tile([P, DIM], f32)
        nc.gpsimd.indirect_dma_start(
            out=q_sb[:], out_offset=None,
            in_=embed_ap[:],
            in_offset=IndirectOffsetOnAxis(ap=idx_i[:, 0:1], axis=0),
        )
        nc.sync.dma_start(quant_ap[tok, :], q_sb[:])

    # ---------- write indices ----------
    # idx_acc [128 tokens-in-tile, nt tiles] -> transpose 128-column chunks so
    # DRAM writes are contiguous: out row = tile index, 128 int32 per row.
    for h in range((nt + P - 1) // P):
        cols = min(P, nt - h * P)
        tr_ps = psum_x.tile([P, P], f32, tag="xt_ps")
        nc.tensor.transpose(tr_ps[:cols, :], idx_acc[:, h * P:h * P + cols],
                            identity[:])
        tr_i32 = p_s.tile([P, P], mybir.dt.int32, tag="idx_i32")
        nc.vector.tensor_scalar(out=tr_i32[:cols, :], in0=tr_ps[:cols, :],
                                scalar1=-1.0, scalar2=None,
                                op0=mybir.AluOpType.add)
        nc.sync.dma_start(idx_view[h * P:h * P + cols, :], tr_i32[:cols, :])

    if rep_cm is not None:
        rep_cm.__exit__(None, None, None)

    ctx.close()


def build_nc(n_tokens=32768, repeat=1):
    nc = bacc.Bacc("TRN2", target_bir_lowering=False, debug=False,
                   enable_asserts=False, num_devices=N_CORES)
    x = nc.dram_tensor("x", [n_tokens, DIM], mybir.dt.float32,
                       kind="ExternalInput").ap()
    em = nc.dram_tensor("embed", [BINS, DIM], mybir.dt.float32,
                        kind="ExternalInput").ap()
    qu = nc.dram_tensor("quant", [n_tokens, DIM], mybir.dt.float32,
                        kind="ExternalOutput").ap()
    ix = nc.dram_tensor("embed_ind", [n_tokens], mybir.dt.int32,
                        kind="ExternalOutput").ap()
    with tile.TileContext(nc) as tc:
        build_vq_body(tc, x, em, qu, ix, n_tokens, repeat=repeat)
    nc.compile()
    return nc


_CACHED_NC = None


def _get_nc():
    global _CACHED_NC
    if _CACHED_NC is None:
        _CACHED_NC = build_nc()
    return _CACHED_NC


def run_sharded(hidden_states, embed, trace=False):
    """Run on 8 cores. Returns (quantize, embed_ind, exec_time_ns|None)."""
    from concourse import bass_utils

    hs = np.ascontiguousarray(np.asarray(hidden_states, dtype=np.float32))
    em = np.ascontiguousarray(np.asarray(embed, dtype=np.float32))
    B, T, D = hs.shape
    flat = hs.reshape(-1, D)
    n_per = flat.shape[0] // N_CORES
    nc = _get_nc()
    in_maps = [{"x": flat[i * n_per:(i + 1) * n_per], "embed": em}
               for i in range(N_CORES)]
    res = bass_utils.run_bass_kernel_spmd(
        nc, in_maps, core_ids=list(range(N_CORES)), trace=trace)
    quant = np.concatenate([res.results[i]["quant"] for i in range(N_CORES)],
                           axis=0).reshape(B, T, D)
    idx = np.concatenate(
        [res.results[i]["embed_ind"] for i in range(N_CORES)],
        axis=0).reshape(B, T).astype(np.int32)
    return quant, idx, res.exec_time_ns


def kernel(hidden_states, embed):
    quant, idx, _ = run_sharded(hidden_states, embed, trace=False)
    return quant, idx


# revision 20
# speedup vs baseline: 1.1945x; 1.1945x over previous
"""Trainium2 Bass kernel for EncodecEuclideanCodebook (VQ encode+decode).

Problem: hidden_states [8, 32768, 128] f32, embed [1024, 128] f32.
  scores = x @ E^T - ||e||^2/2   (argmax-equivalent to reference dist)
  embed_ind = argmax(scores, axis=-1)           -> [8, 32768] int32
  quantize  = embed[embed_ind]                  -> [8, 32768, 128] f32

Sharding: data-parallel over tokens; core i handles hidden_states[i]
(32768 tokens). embed replicated. No collectives.

Per-core pipeline, 256 tiles of 128 tokens:
  DMA x tile -> PE transpose (x^T) -> fp32 matmuls vs resident E^T
  (+ bf16x3-decomposed -e^2/2 bias row matmul) -> scores in PSUM
  -> ACT copy PSUM->SBUF -> GPSIMD max-reduce (or DVE InstMax for a
  fraction of tiles, to balance engines) -> DVE max_index -> idx
  -> GPSIMD indirect-DMA gather embed[idx] -> DMA out.

Exactness notes (measured on the seed-0 data offline):
  min top-2 score gap = 2.1e-5, so bf16/tf32 GEMM or any quantized
  argmax packing flips indices; full-fp32 GEMM and exact fp32
  compares are required. jnp.argmax picks the first max; InstMaxIndex
  also returns the first match.
"""

import os
import sys

sys.path.insert(0, "/opt/trn_rl_repo")

import numpy as np

import concourse.bass as bass
import concourse.mybir as mybir
import concourse.tile as tile
from concourse import bacc
from concourse.bass import IndirectOffsetOnAxis
from concourse.masks import make_identity

# ---------------------------------------------------------------------------
# Custom DVE ops (registered into concourse's OPS table at import time).
#
# VQ_PAIRMAX_RED: out[k] = max(in0[k], in1[k]); accum_out = row max.
#   One 512-elem pass examines all 1024 scores (2 streams) and yields the
#   exact fp32 row max.
# VQ_ARGMIN_HIT: cand[k] = (k+1) + (in0[k] < m) * BIG; accum_out = min cand
#   = 1 + first index where in0 == m (first occurrence, matching jnp.argmax).
#   BIG = 1e9 via s1 (also the accum seed).
# ---------------------------------------------------------------------------
from concourse import dve_ops as _dve_ops
from concourse.dve_spec import (
    Spec as _Spec, Src0 as _Src0, Src1 as _Src1, C0 as _C0, C1 as _C1,
    maxx as _maxx, minn as _minn, scan as _scan, AluOp as _AluOp, One as _One,
    lower as _lower,
)
from concourse.dve_uop import DveOpSpec as _DveOpSpec
import numpy as _np


def _ref_pairmax(in0, in1, c0, c1, c2):
    b = _np.maximum(in0, in1).astype(_np.float32)
    return b, b.reshape(b.shape[0], -1).max(axis=-1, keepdims=True)


def _ref_argmin_hit(in0, in1, c0, c1, c2):
    n = in0.shape[-1]
    pos = _np.arange(1, n + 1, dtype=_np.float32).reshape(
        (1,) * (in0.ndim - 1) + (n,))
    cand = (pos + (in0 < c0) * c1).astype(_np.float32)
    flat = cand.reshape(cand.shape[0], -1)
    acc = _np.minimum(flat.min(axis=-1, keepdims=True),
                      _np.asarray(c1, _np.float32).reshape(-1, 1)
                      if isinstance(c1, _np.ndarray) else c1)
    return cand, acc


def _register_op(name, spec, rd1_en):
    existing = {o.name: o for o in _dve_ops.OPS}
    if name in existing:
        return existing[name]
    op = _dve_ops.DveOp(name, spec, subdim=False, uops_sha={})
    _dve_ops.OPS.append(op)
    _dve_ops.CUSTOM_DVE_SPECS[name] = spec
    _dve_ops._SUB_OPCODE_FOR_NAME[name] = (
        _dve_ops._CUSTOM_DVE_ROW_BASE + len(_dve_ops.OPS) - 1)
    assert _dve_ops._SUB_OPCODE_FOR_NAME[name] < 0x20
    shas = {}
    for ver in ("v3", "v4"):
        u = _lower(spec, ver=ver)
        shas[ver] = _DveOpSpec(name=name, opcode=_dve_ops.get_dve_sub_opcode(name),
                               uops=u, rd1_en=rd1_en).sha(ver)
    object.__setattr__(op, "uops_sha", shas)
    return op


VQ_PAIRMAX_RED = _register_op(
    "VQ_PAIRMAX_RED",
    _Spec(body=_maxx(_Src0, _Src1), accum=_maxx, reference=_ref_pairmax),
    rd1_en=True)

VQ_ARGMIN_HIT = _register_op(
    "VQ_ARGMIN_HIT",
    _Spec(body=_scan(_AluOp.ADD, _One) + (_Src0 < _C0) * _C1,
          accum=_minn, accum_init=_C1, reference=_ref_argmin_hit),
    rd1_en=False)

P = 128
BINS = 1024
DIM = 128
N_CORES = 8

# Tiles with (t % DVE_MAX_EVERY == DVE_MAX_EVERY-1) run the row-max fully on
# DVE (InstMax over 1024); the rest use GPSIMD pairwise-max + DVE InstMax over
# 512. Balances scan work between DVE and GPSIMD. 0 -> all GPSIMD-assisted.
DVE_MAX_EVERY = 0


def build_vq_body(tc, x_ap, embed_ap, quant_ap, idx_ap, n_tokens, repeat=1):
    """Emit the per-core kernel body. x [n_tokens,128] f32, embed [1024,128]
    f32, quant [n_tokens,128] f32, idx [n_tokens] int32 (DRAM APs).
    repeat>1 wraps the main loop in a hardware For_i that redoes the whole
    token sweep (idempotent) -- used only for wall-clock timing deltas."""
    nc = tc.nc
    nt = n_tokens // P  # token tiles
    f32 = mybir.dt.float32
    bf16 = mybir.dt.bfloat16

    from contextlib import ExitStack
    ctx = ExitStack()
    const = ctx.enter_context(tc.tile_pool(name="const", bufs=1))

    # ---------- one-time setup (own PSUM stack, freed before main loop) ----
    setup_ctx = ExitStack()
    psum_su = setup_ctx.enter_context(
        tc.tile_pool(name="psum_su", bufs=1, space="PSUM"))

    identity = const.tile([P, P], f32)
    make_identity(nc, identity[:])

    # E^T resident in SBUF: [dim=128 partitions, 1024 bins]
    et = const.tile([P, BINS], f32)
    for k in range(BINS // P):
        e_blk = const.tile([P, DIM], f32, tag="e_blk")
        nc.sync.dma_start(e_blk[:], embed_ap[k * P:(k + 1) * P, :])
        et_ps = psum_su.tile([P, P], f32, tag="setup_ps")
        nc.tensor.transpose(et_ps[:], e_blk[:], identity[:])
        nc.scalar.copy(et[:, k * P:(k + 1) * P], et_ps[:])

    # e_sq column sums via ones^T @ (E^T * E^T)
    sq = const.tile([P, BINS], f32)
    nc.vector.tensor_tensor(out=sq[:], in0=et[:], in1=et[:],
                            op=mybir.AluOpType.mult)
    ones_col = const.tile([P, 1], f32)
    nc.vector.memset(ones_col[:], 1.0)
    es_sb = const.tile([1, BINS], f32)
    for h in range(2):
        sl = slice(h * 512, (h + 1) * 512)
        es_ps = psum_su.tile([1, 512], f32, tag="es_ps")
        nc.tensor.matmul(es_ps[:], lhsT=ones_col[:], rhs=sq[:, sl],
                         start=True, stop=True)
        nc.scalar.copy(es_sb[0:1, sl], es_ps[:])
    # bias = -e_sq/2, decomposed into 3 bf16 rows (hi+mid+lo ~ exact fp32)
    b_f32 = const.tile([1, BINS], f32)
    nc.vector.tensor_scalar(out=b_f32[:], in0=es_sb[:], scalar1=-0.5,
                            scalar2=None, op0=mybir.AluOpType.mult)
    hi = const.tile([1, BINS], bf16)
    nc.vector.tensor_copy(hi[:], b_f32[:])
    r1 = const.tile([1, BINS], f32)
    nc.vector.tensor_tensor(out=r1[:], in0=b_f32[:], in1=hi[:],
                            op=mybir.AluOpType.subtract)
    mid = const.tile([1, BINS], bf16)
    nc.vector.tensor_copy(mid[:], r1[:])
    r2 = const.tile([1, BINS], f32)
    nc.vector.tensor_tensor(out=r2[:], in0=r1[:], in1=mid[:],
                            op=mybir.AluOpType.subtract)
    lo = const.tile([1, BINS], bf16)
    nc.vector.tensor_copy(lo[:], r2[:])
    bias3 = const.tile([3, BINS], bf16)
    nc.sync.dma_start(bias3[0:1, :], hi[:])
    nc.sync.dma_start(bias3[1:2, :], mid[:])
    nc.sync.dma_start(bias3[2:3, :], lo[:])
    ones3 = const.tile([3, P], bf16)
    nc.vector.memset(ones3[:], 1.0)

    setup_ctx.close()  # release setup PSUM banks

    psum_x = ctx.enter_context(tc.tile_pool(name="psum_x", bufs=2, space="PSUM"))
    psum_s = ctx.enter_context(tc.tile_pool(name="psum_s", bufs=3, space="PSUM"))
    p_xa = ctx.enter_context(tc.tile_pool(name="xa", bufs=3))
    p_xt = ctx.enter_context(tc.tile_pool(name="xt", bufs=4))
    p_s = ctx.enter_context(tc.tile_pool(name="s", bufs=3))
    p_m = ctx.enter_context(tc.tile_pool(name="m", bufs=6))
    p_i = ctx.enter_context(tc.tile_pool(name="i", bufs=3))
    p_q = ctx.enter_context(tc.tile_pool(name="q", bufs=3))

    # idx accumulator: column t holds tile t's indices (as f32, exact ints)
    idx_acc = const.tile([P, nt], f32)

    idx_view = idx_ap.rearrange("(t p) -> t p", p=P)  # [nt, 128]

    GB = 4 if nt % 4 == 0 else (2 if nt % 2 == 0 else 1)  # tiles per DMA group

    rep_cm = tc.For_i(0, repeat, 1) if repeat > 1 else None
    if rep_cm is not None:
        rep_cm.__enter__()

    # ---------- main loop ----------
    for g in range(nt // GB):
        # batched x load: GB*128 consecutive tokens -> [128, GB, 128]
        xg = p_xa.tile([P, GB, DIM], f32)
        nc.sync.dma_start(
            xg[:], x_ap[g * GB * P:(g + 1) * GB * P, :]
            .rearrange("(j p) d -> p j d", p=P))
        idx_g = p_i.tile([P, GB], mybir.dt.int32)
        q_g = p_q.tile([P, GB, DIM], f32)

        # all GB transposes into one PSUM bank, one batched ACT copy out --
        # keeps ACT's xt work off the per-tile mm -> b1 critical loop
        xt_ps = psum_x.tile([P, GB * P], f32, tag="xt_ps")
        for j in range(GB):
            nc.tensor.transpose(xt_ps[:, j * P:(j + 1) * P], xg[:, j, :],
                                identity[:])
        xt_g = p_xt.tile([P, GB * P], f32)
        nc.scalar.copy(xt_g[:], xt_ps[:])

        for j in range(GB):
            t = g * GB + j
            xt = xt_g[:, j * P:(j + 1) * P]

            s_ps = psum_s.tile([P, BINS], f32)
            for h in range(2):
                sl = slice(h * 512, (h + 1) * 512)
                nc.tensor.matmul(s_ps[:, sl], lhsT=xt, rhs=et[:, sl],
                                 start=True, stop=False)
                nc.tensor.matmul(s_ps[:, sl], lhsT=ones3[:], rhs=bias3[:, sl],
                                 start=False, stop=True)

            # Row max: ACT copies bank1 to SBUF so the custom pairmax op can
            # stream PSUM bank0 + SBUF bank1 (one pass over 512 pairs).
            b1_sb = p_s.tile([P, 512], f32, tag="b1")
            nc.scalar.copy(b1_sb[:], s_ps[:, 512:1024])
            pm = p_s.tile([P, 512], f32, tag="pm")
            m1 = p_m.tile([P, 1], f32, tag="m1")
            nc.vector._custom_dve(VQ_PAIRMAX_RED, out=pm[:], accum_out=m1[:],
                                  in0=s_ps[:, 0:512], in1=b1_sb[:])
            # First index where score == max: min over (pos+1 | miss*1e9).
            am = p_s.tile([P, BINS], f32, tag="am")
            idxp1 = p_m.tile([P, 1], f32, tag="idxp1")
            nc.vector._custom_dve(VQ_ARGMIN_HIT, out=am[:], accum_out=idxp1[:],
                                  in0=s_ps[:, 0:BINS], s0=m1[:, 0:1], s1=1e9)
            nc.vector.tensor_copy(idx_acc[:, t:t + 1], idxp1[:, 0:1])  # idx+1
            nc.vector.tensor_scalar(out=idx_g[:, j:j + 1], in0=idxp1[:],
                                    scalar1=-1.0, scalar2=None,
                                    op0=mybir.AluOpType.add)
            # per-tile gather (multi-offset indirect DMA corrupts on HW)
            nc.gpsimd.indirect_dma_start(
                out=q_g[:, j, :], out_offset=None,
                in_=embed_ap[:],
                in_offset=IndirectOffsetOnAxis(ap=idx_g[:, j:j + 1], axis=0),
            )

        # batched store for the group
        nc.sync.dma_start(
            quant_ap[g * GB * P:(g + 1) * GB * P, :]
            .rearrange("(j p) d -> p j d", p=P), q_g[:])

    # ---------- write indices ----------
    # idx_acc [128 tokens-in-tile, nt tiles] -> transpose 128-column chunks so
    # DRAM writes are contiguous: out row = tile index, 128 int32 per row.
    for h in range((nt + P - 1) // P):
        cols = min(P, nt - h * P)
        tr_ps = psum_x.tile([P, P], f32, tag="xt_ps")
        nc.tensor.transpose(tr_ps[:cols, :], idx_acc[:, h * P:h * P + cols],
                            identity[:])
        tr_i32 = p_s.tile([P, P], mybir.dt.int32, tag="idx_i32")
        nc.vector.tensor_scalar(out=tr_i32[:cols, :], in0=tr_ps[:cols, :],
                                scalar1=-1.0, scalar2=None,
                                op0=mybir.AluOpType.add)
        nc.sync.dma_start(idx_view[h * P:h * P + cols, :], tr_i32[:cols, :])

    if rep_cm is not None:
        rep_cm.__exit__(None, None, None)

    ctx.close()


def build_nc(n_tokens=32768, repeat=1):
    nc = bacc.Bacc("TRN2", target_bir_lowering=False, debug=False,
                   enable_asserts=False, num_devices=N_CORES)
    x = nc.dram_tensor("x", [n_tokens, DIM], mybir.dt.float32,
                       kind="ExternalInput").ap()
    em = nc.dram_tensor("embed", [BINS, DIM], mybir.dt.float32,
                        kind="ExternalInput").ap()
    qu = nc.dram_tensor("quant", [n_tokens, DIM], mybir.dt.float32,
                        kind="ExternalOutput").ap()
    ix = nc.dram_tensor("embed_ind", [n_tokens], mybir.dt.int32,
                        kind="ExternalOutput").ap()
    with tile.TileContext(nc) as tc:
        build_vq_body(tc, x, em, qu, ix, n_tokens, repeat=repeat)
    nc.compile()
    return nc


_CACHED_NC = None


def _get_nc():
    global _CACHED_NC
    if _CACHED_NC is None:
        _CACHED_NC = build_nc()
    return _CACHED_NC


def run_sharded(hidden_states, embed, trace=False):
    """Run on 8 cores. Returns (quantize, embed_ind, exec_time_ns|None)."""
    from concourse import bass_utils

    hs = np.ascontiguousarray(np.asarray(hidden_states, dtype=np.float32))
    em = np.ascontiguousarray(np.asarray(embed, dtype=np.float32))
    B, T, D = hs.shape
    flat = hs.reshape(-1, D)
    n_per = flat.shape[0] // N_CORES
    nc = _get_nc()
    in_maps = [{"x": flat[i * n_per:(i + 1) * n_per], "embed": em}
               for i in range(N_CORES)]
    res = bass_utils.run_bass_kernel_spmd(
        nc, in_maps, core_ids=list(range(N_CORES)), trace=trace)
    quant = np.concatenate([res.results[i]["quant"] for i in range(N_CORES)],
                           axis=0).reshape(B, T, D)
    idx = np.concatenate(
        [res.results[i]["embed_ind"] for i in range(N_CORES)],
        axis=0).reshape(B, T).astype(np.int32)
    return quant, idx, res.exec_time_ns


def kernel(hidden_states, embed):
    quant, idx, _ = run_sharded(hidden_states, embed, trace=False)
    return quant, idx
